# revision 3
# baseline (speedup 1.0000x reference)
"""Trainium2 Bass kernel for nn_DeRNN_4054449127979.

Network (per reference):
  stage1: 6 shared-weight single-channel LSTMs (hidden 16) over T=1024,
          folded as one LSTM on [B*6, T, 1]; keep last hidden -> feat [B, 96]
  stage2: LSTM(1 -> 128) over the 96 features as a sequence (return_seq)
  stage3: LSTM(128 -> 128) over those 96 steps; keep last hidden
  head:   relu(fc1) -> relu(fc3) -> fc2  -> [B, 2]

Sharding: pure data parallel over batch across 8 cores (B=2048 -> 256/core).
No collectives (inference, params replicated per core).

stage1 v2: two interleaved batch half-streams (128 each) to fill the
recurrence-latency bubbles. State tile per stream = [h(96) | x(6) | ones]
= 103 rows, so each gate is ONE stacked matmul [103 -> 96] (weights carry
Whh block-diag + Wih + bias): 8 matmuls/step-pair instead of 16, emitted
same-gate-adjacent so weight reloads can coalesce. Forget-gate product and
the per-step x-row copy run on the (otherwise idle) GpSimd engine; sigma is
split (i,f early / o off-path) to shorten the critical chain.
stage2/3: hidden 128 on partitions, 256 batch on free, wavefronted steps.
All matmuls float32r (TF32); cell math fp32.
"""

import sys

import numpy as np

sys.path.insert(0, "/opt/trn_rl_repo")

import concourse.bass as bass  # noqa: E402
import concourse.tile as tile  # noqa: E402
from concourse import bacc, mybir  # noqa: E402

F32 = mybir.dt.float32
F32R = mybir.dt.float32r
AF = mybir.ActivationFunctionType
ALU = mybir.AluOpType

B = 2048
NCORES = 8
BC = B // NCORES  # 256
BH = BC // 2  # 128 per stream
NCH = 6
H1 = 16
G1 = NCH * H1  # 96
H2 = 128
T1_FULL = 1024
T2_FULL = 96
CH = 32  # stage-1 steps per staged x chunk
NST = G1 + NCH + 1  # 103 state rows: h | x | ones

# on-chip gate bank order: i, f, o, g ; torch rows are i, f, g, o
GATE_BASES_1 = (0, H1, 3 * H1, 2 * H1)
GATE_BASES_2 = (0, H2, 3 * H2, 2 * H2)

# packed weight column map
_off = 0


def _take(n):
    global _off
    o = _off
    _off += n
    return o


W1S_O = _take(4 * G1)  # stacked stage-1 weights, rows 0:103
W2X_O = _take(4 * H2)  # row 0 (x weights)
B2_O = _take(4 * H2)  # row 0 (stage-2 bias)
W2H_O = _take(4 * H2)
W3X_O = _take(4 * H2)
W3H_O = _take(4 * H2)
B3_O = _take(4 * H2)
WF1_O = _take(H2)
WF3_O = _take(H2)
WF2_O = _take(2)
BF1_O = _take(1)
BF3_O = _take(1)
BF2_O = _take(1)
ONES_O = _take(BC)
WCOLS = _off


def _r(ap, pattern, **kw):
    return ap.rearrange(pattern, **kw)


def build_program(T1=T1_FULL, T2=T2_FULL, ch=CH, staggered=False):
    assert T1 % (2 * ch) == 0
    nc = bacc.Bacc("TRN2", target_bir_lowering=False)

    tpad = 2 * ch
    x_t = nc.declare_dram_parameter("xT", [7, T1 + tpad, BC], F32R, isOutput=False)
    wp_d = nc.declare_dram_parameter("wpack", [128, WCOLS], F32R, isOutput=False)
    y_t = nc.declare_dram_parameter("yT", [2, BC], F32, isOutput=True)

    feat_d = nc.dram_tensor("featstage", [G1 + 32, BC], F32R)

    def mm(out, lhsT, rhs, start, stop):
        nc.tensor.matmul(out, lhsT, rhs, start=start, stop=stop)

    with tile.TileContext(nc) as tc:
        with (
            tc.tile_pool(name="wpool", bufs=1) as wpool,
            tc.tile_pool(name="state", bufs=1) as state,
            tc.tile_pool(name="work", bufs=3) as work,
        ):
            wp = wpool.tile([128, WCOLS], F32R)
            nc.sync.dma_start(wp[:], wp_d[:])

            w1s = [wp[0:NST, W1S_O + G1 * t : W1S_O + G1 * (t + 1)] for t in range(4)]
            w2x = [wp[0:1, W2X_O + H2 * t : W2X_O + H2 * (t + 1)] for t in range(4)]
            b2 = [wp[0:1, B2_O + H2 * t : B2_O + H2 * (t + 1)] for t in range(4)]
            w2h = [wp[0:H2, W2H_O + H2 * t : W2H_O + H2 * (t + 1)] for t in range(4)]
            w3x = [wp[0:H2, W3X_O + H2 * t : W3X_O + H2 * (t + 1)] for t in range(4)]
            w3h = [wp[0:H2, W3H_O + H2 * t : W3H_O + H2 * (t + 1)] for t in range(4)]
            b3 = [wp[0:1, B3_O + H2 * t : B3_O + H2 * (t + 1)] for t in range(4)]
            wf1 = wp[0:H2, WF1_O : WF1_O + H2]
            wf3 = wp[0:H2, WF3_O : WF3_O + H2]
            wf2 = wp[0:H2, WF2_O : WF2_O + 2]
            bf1 = wp[0:H2, BF1_O : BF1_O + 1].bitcast(F32)
            bf3 = wp[0:H2, BF3_O : BF3_O + 1].bitcast(F32)
            bf2 = wp[0:2, BF2_O : BF2_O + 1].bitcast(F32)
            ones1 = wp[0:1, ONES_O : ONES_O + BC]

            # ---- stage 1: two interleaved half-streams ----
            st = [
                [state.tile([NST, BH], F32R, name=f"st{s}_{p}") for p in range(2)]
                for s in range(2)
            ]
            c1 = [state.tile([G1, BH], F32, name=f"c1_{s}") for s in range(2)]
            for s in range(2):
                for p in range(2):
                    nc.vector.memset(st[s][p][:].bitcast(F32), 0.0)
                    # rows 96:103 -> 1.0 (aligned start); x copies later
                    # overwrite 96:102, leaving the ones row at 102
                    nc.vector.memset(st[s][p][G1:NST, :].bitcast(F32), 1.0)
                nc.vector.memset(c1[s][:], 0.0)

            with (
                tc.tile_pool(name="ps1pool", bufs=1, space="PSUM") as ps1pool,
                tc.tile_pool(name="xsb", bufs=1) as xsb,
            ):
                xstage = [
                    xsb.tile([NST, ch * BC], F32R, name=f"xst_{k}") for k in range(2)
                ]
                xv = _r(x_t[:], "c t b -> c (t b)")
                nc.sync.dma_start(
                    xstage[0][G1 : G1 + NCH, :], xv[0:NCH, 0 : ch * BC]
                )
                nc.sync.dma_start(
                    xstage[1][G1 : G1 + NCH, :], xv[0:NCH, ch * BC : 2 * ch * BC]
                )
                nchunks = T1 // ch

                def xview(k, jj, s):
                    return _r(xstage[k], "p (t b) -> p t b", b=BC)[
                        G1 : G1 + NCH, jj, s * BH : (s + 1) * BH
                    ]

                # prologue: x_0 into both ping-0 state tiles
                for s in range(2):
                    nc.gpsimd.tensor_copy(st[s][0][G1 : G1 + NCH, :], xview(0, 0, s))

                def pair_step(k, jj, p, knext, jjnext):
                    ps = [
                        ps1pool.tile(
                            [G1, 2048], F32, name=f"ps1{s}", tag=f"ps1{s}"
                        )
                        for s in range(2)
                    ]
                    for t in (3, 0, 1, 2):  # g first so tanh starts early
                        for s in range(2):
                            mm(
                                ps[s][:, t * 512 : t * 512 + BH],
                                w1s[t],
                                st[s][p][:],
                                True,
                                True,
                            )
                    gt, sif, so, th, u, tm = [], [], [], [], [], []
                    for s in range(2):
                        g_ = work.tile([G1, BH], F32, name=f"gt{s}", tag="gt")
                        nc.scalar.activation(
                            g_[:], ps[s][:, 3 * 512 : 3 * 512 + BH], AF.Tanh
                        )
                        sif_ = work.tile([G1, 2 * BH], F32, name=f"sif{s}", tag="sif")
                        nc.scalar.activation(
                            _r(sif_, "p (t b) -> p t b", b=BH),
                            _r(ps[s][:, :], "p (t b) -> p t b", b=512)[:, 0:2, 0:BH],
                            AF.Sigmoid,
                        )
                        gt.append(g_)
                        sif.append(sif_)
                    for s in range(2):
                        so_ = work.tile([G1, BH], F32, name=f"so{s}", tag="so")
                        nc.scalar.activation(
                            so_[:], ps[s][:, 2 * 512 : 2 * 512 + BH], AF.Sigmoid
                        )
                        so.append(so_)
                    for s in range(2):
                        u_ = work.tile([G1, BH], F32, name=f"u{s}", tag="u")
                        nc.vector.tensor_mul(u_[:], sif[s][:, 0:BH], gt[s][:])
                        tm_ = work.tile([G1, BH], F32, name=f"tm{s}", tag="tm")
                        nc.gpsimd.tensor_mul(tm_[:], sif[s][:, BH:], c1[s][:])
                        nc.vector.tensor_add(c1[s][:], u_[:], tm_[:])
                    for s in range(2):
                        th_ = work.tile([G1, BH], F32, name=f"th{s}", tag="th")
                        nc.scalar.activation(th_[:], c1[s][:], AF.Tanh)
                        th.append(th_)
                    for s in range(2):
                        nc.vector.tensor_mul(
                            st[s][1 - p][0:G1, :], so[s][:], th[s][:]
                        )
                        nc.gpsimd.tensor_copy(
                            st[s][1 - p][G1 : G1 + NCH, :], xview(knext, jjnext, s)
                        )

                def chunk_pair(ivc):
                    for k in range(2):
                        for jj in range(ch):
                            if jj < ch - 1:
                                kn, jn = k, jj + 1
                            else:
                                kn, jn = 1 - k, 0
                            pair_step(k, jj, jj % 2, kn, jn)
                        pre = (ivc + (2 + k)) * (ch * BC)
                        nc.sync.dma_start(
                            xstage[k][G1 : G1 + NCH, :],
                            xv[0:NCH, bass.ds(pre, ch * BC)],
                        )

                chunk_pair(0)  # peeled: absorbs prologue DMA waits
                assert nchunks >= 4
                with tc.For_i(2, nchunks, 2, staggered_reset=staggered) as ivc:
                    chunk_pair(ivc)

                # final h -> feat  (last write was into ping T1%2)
                pf = T1 % 2
                nc.sync.dma_start(feat_d[0:G1, 0:BH], st[0][pf][0:G1, :])
                nc.sync.dma_start(feat_d[0:G1, BH:BC], st[1][pf][0:G1, :])

            # ---- stages 2 & 3, wavefronted ----
            with tc.tile_pool(name="psum", bufs=2, space="PSUM") as psum_pool:
                h2s = [state.tile([H2, BC], F32R, name=f"h2s_{p}") for p in range(2)]
                c2 = state.tile([H2, BC], F32)
                h3s = [state.tile([H2, BC], F32R, name=f"h3s_{p}") for p in range(2)]
                c3 = state.tile([H2, BC], F32)
                for t_ in h2s + h3s:
                    nc.vector.memset(t_[:].bitcast(F32), 0.0)
                nc.vector.memset(c2[:], 0.0)
                nc.vector.memset(c3[:], 0.0)
                x2blk = [
                    state.tile([1, 8 * BC], F32R, name=f"x2blk_{p}") for p in range(2)
                ]

                def cell23(ps, c, h_out, pfx):
                    gt = work.tile([H2, BC], F32, name=f"gt{pfx}", tag="gt23")
                    nc.scalar.activation(gt[:], ps[:, 3 * 512 : 3 * 512 + BC], AF.Tanh)
                    s_all = work.tile([H2, 3 * BC], F32, name=f"s{pfx}", tag="s23")
                    nc.scalar.activation(
                        _r(s_all, "p (t b) -> p t b", b=BC),
                        _r(ps, "p (t b) -> p t b", b=512)[:, 0:3, 0:BC],
                        AF.Sigmoid,
                    )
                    u = work.tile([H2, BC], F32, name=f"u{pfx}", tag="u23")
                    nc.vector.tensor_mul(u[:], s_all[:, 0:BC], gt[:])
                    tm = work.tile([H2, BC], F32, name=f"tm{pfx}", tag="tm23")
                    nc.gpsimd.tensor_mul(tm[:], s_all[:, BC : 2 * BC], c[:])
                    nc.vector.tensor_add(c[:], u[:], tm[:])
                    th = work.tile([H2, BC], F32, name=f"th{pfx}", tag="th23")
                    nc.scalar.activation(th[:], c[:], AF.Tanh)
                    nc.vector.tensor_mul(h_out[:], s_all[:, 2 * BC :], th[:])

                def step2(j, xrow):
                    ps = psum_pool.tile([128, 2048], F32, name="ps2", tag="ps")
                    for t in range(4):
                        mm(ps[:, t * 512 : t * 512 + BC], b2[t], ones1, True, False)
                    for t in range(4):
                        mm(ps[:, t * 512 : t * 512 + BC], w2x[t], xrow, False, False)
                    for t in (3, 0, 1, 2):
                        mm(
                            ps[:, t * 512 : t * 512 + BC],
                            w2h[t],
                            h2s[j % 2],
                            False,
                            True,
                        )
                    cell23(ps, c2, h2s[(j + 1) % 2], "2")

                def step3(j):
                    ps = psum_pool.tile([128, 2048], F32, name="ps3", tag="ps")
                    for t in range(4):
                        mm(ps[:, t * 512 : t * 512 + BC], b3[t], ones1, True, False)
                    for t in range(4):
                        mm(
                            ps[:, t * 512 : t * 512 + BC],
                            w3x[t],
                            h2s[(j + 1) % 2],
                            False,
                            False,
                        )
                    for t in (3, 0, 1, 2):
                        mm(ps[:, t * 512 : t * 512 + BC], w3h[t], h3s[j % 2], False, True)
                    cell23(ps, c3, h3s[(j + 1) % 2], "3")

                W2 = 16
                nc.sync.dma_start(x2blk[0][:], _r(feat_d[0:8, :], "r b -> (r b)"))
                nc.sync.dma_start(x2blk[1][:], _r(feat_d[8:16, :], "r b -> (r b)"))

                def wave_block(ivw):
                    for half in range(2):
                        for jj in range(8):
                            j = 8 * half + jj
                            step2(j, x2blk[half][0:1, jj * BC : (jj + 1) * BC])
                            step3(j)
                        off = ivw + 16 + 8 * half
                        nc.sync.dma_start(
                            x2blk[half][:],
                            _r(feat_d[bass.ds(off, 8), :], "r b -> (r b)"),
                        )

                wave_block(0)  # peeled
                with tc.For_i(W2, T2, W2, staggered_reset=staggered) as ivw:
                    wave_block(ivw)

                # ---- FC head ----
                h3f = h3s[T2 % 2]
                psf = psum_pool.tile([128, 2048], F32, name="psf", tag="ps")
                mm(psf[:, 0:BC], wf1, h3f, True, True)
                a1 = work.tile([H2, BC], F32R)
                nc.vector.tensor_scalar(
                    a1[:], psf[:, 0:BC], bf1, 0.0, op0=ALU.add, op1=ALU.max
                )
                mm(psf[:, 512 : 512 + BC], wf3, a1, True, True)
                a3 = work.tile([H2, BC], F32R)
                nc.vector.tensor_scalar(
                    a3[:], psf[:, 512 : 512 + BC], bf3, 0.0, op0=ALU.add, op1=ALU.max
                )
                mm(psf[0:2, 1024 : 1024 + BC], wf2, a3, True, True)
                yt = work.tile([2, BC], F32)
                nc.vector.tensor_scalar_add(yt[:], psf[0:2, 1024 : 1024 + BC], bf2)
                nc.sync.dma_start(y_t[:], yt[:])

    nc.compile()
    return nc


def tf32_round(a):
    u = np.ascontiguousarray(a, np.float32).view(np.uint32)
    u = (u + np.uint32(0x1000)) & np.uint32(0xFFFFE000)
    return u.view(np.float32)


def pack_weights(i):
    f32 = np.float32
    wp = np.zeros((128, WCOLS), f32)
    Wih, Whh = np.asarray(i["rnn_Wih"], f32), np.asarray(i["rnn_Whh"], f32)
    bb1 = np.asarray(i["rnn_bih"], f32) + np.asarray(i["rnn_bhh"], f32)
    for t, base in enumerate(GATE_BASES_1):
        o = W1S_O + G1 * t
        for c in range(NCH):
            # h-part rows 0:96 (block-diagonal per channel)
            wp[16 * c : 16 * c + 16, o + 16 * c : o + 16 * c + 16] = (
                Whh[base : base + H1, :].T
            )
            # x-part row 96+c
            wp[G1 + c, o + 16 * c : o + 16 * c + 16] = Wih[base : base + H1, 0]
        # bias row 102
        wp[G1 + NCH, o : o + G1] = np.tile(bb1[base : base + H1], NCH)
    bb2 = np.asarray(i["rnn2_bih0"], f32) + np.asarray(i["rnn2_bhh0"], f32)
    for t, base in enumerate(GATE_BASES_2):
        wp[0, W2X_O + H2 * t : W2X_O + H2 * (t + 1)] = np.asarray(
            i["rnn2_Wih0"], f32
        )[base : base + H2, 0]
        wp[0, B2_O + H2 * t : B2_O + H2 * (t + 1)] = bb2[base : base + H2]
        wp[0:H2, W2H_O + H2 * t : W2H_O + H2 * (t + 1)] = np.asarray(
            i["rnn2_Whh0"], f32
        )[base : base + H2, :].T
    bb3 = np.asarray(i["rnn2_bih1"], f32) + np.asarray(i["rnn2_bhh1"], f32)
    for t, base in enumerate(GATE_BASES_2):
        wp[0:H2, W3X_O + H2 * t : W3X_O + H2 * (t + 1)] = np.asarray(
            i["rnn2_Wih1"], f32
        )[base : base + H2, :].T
        wp[0:H2, W3H_O + H2 * t : W3H_O + H2 * (t + 1)] = np.asarray(
            i["rnn2_Whh1"], f32
        )[base : base + H2, :].T
        wp[0, B3_O + H2 * t : B3_O + H2 * (t + 1)] = bb3[base : base + H2]
    wp[0:H2, WF1_O : WF1_O + H2] = np.asarray(i["fc1_W"], f32).T
    wp[0:H2, WF3_O : WF3_O + H2] = np.asarray(i["fc3_W"], f32).T
    wp[0:H2, WF2_O : WF2_O + 2] = np.asarray(i["fc2_W"], f32).T
    # matmul-consumed columns get TF32 rounding
    wp = tf32_round(wp)
    # biases used as fp32 DVE scalars keep full precision
    wp[0:H2, BF1_O] = np.asarray(i["fc1_b"], f32)
    wp[0:H2, BF3_O] = np.asarray(i["fc3_b"], f32)
    wp[0:2, BF2_O] = np.asarray(i["fc2_b"], f32)
    wp[0, ONES_O : ONES_O + BC] = 1.0
    return wp


def make_in_maps(inputs, T1=T1_FULL, ch=CH):
    wp = pack_weights(inputs)
    x = np.asarray(inputs["x"], np.float32)
    tpad = 2 * ch
    maps = []
    for k in range(NCORES):
        xk = np.zeros((7, T1 + tpad, BC), np.float32)
        xk[0:6, :T1, :] = tf32_round(
            np.ascontiguousarray(x[k * BC : (k + 1) * BC, :T1, :].transpose(2, 1, 0))
        )
        xk[6, :, :] = 1.0
        maps.append({"xT": xk, "wpack": wp})
    return maps


def kernel(**inputs):
    from concourse.bass_utils import run_bass_kernel_spmd

    nc = build_program()
    in_maps = make_in_maps(inputs)
    res = run_bass_kernel_spmd(nc, in_maps, list(range(NCORES)))
    outs = [np.asarray(res.results[k]["yT"]) for k in range(NCORES)]
    return np.concatenate([o.T for o in outs], axis=0).astype(np.float32)


# revision 6
# speedup vs baseline: 1.2921x; 1.2921x over previous
"""Trainium2 Bass kernel for nn_DeRNN_4054449127979.

Network (per reference):
  stage1: 6 shared-weight single-channel LSTMs (hidden 16) over T=1024,
          folded as one LSTM on [B*6, T, 1]; keep last hidden -> feat [B, 96]
  stage2: LSTM(1 -> 128) over the 96 features as a sequence (return_seq)
  stage3: LSTM(128 -> 128) over those 96 steps; keep last hidden
  head:   relu(fc1) -> relu(fc3) -> fc2  -> [B, 2]

Sharding: pure data parallel over batch across 8 cores (B=2048 -> 256/core).
No collectives (inference, params replicated per core).

stage1 v2: two interleaved batch half-streams (128 each) to fill the
recurrence-latency bubbles. State tile per stream = [h(96) | x(6) | ones]
= 103 rows, so each gate is ONE stacked matmul [103 -> 96] (weights carry
Whh block-diag + Wih + bias): 8 matmuls/step-pair instead of 16, emitted
same-gate-adjacent so weight reloads can coalesce. Forget-gate product and
the per-step x-row copy run on the (otherwise idle) GpSimd engine; sigma is
split (i,f early / o off-path) to shorten the critical chain.
stage2/3: hidden 128 on partitions, 256 batch on free, wavefronted steps.
All matmuls float32r (TF32); cell math fp32.
"""

import sys

import numpy as np

sys.path.insert(0, "/opt/trn_rl_repo")

import concourse.bass as bass  # noqa: E402
import concourse.tile as tile  # noqa: E402
from concourse import bacc, mybir  # noqa: E402

F32 = mybir.dt.float32
F32R = mybir.dt.float32r
AF = mybir.ActivationFunctionType
ALU = mybir.AluOpType

B = 2048
NCORES = 8
BC = B // NCORES  # 256
BH = BC // 2  # 128 per stream
NCH = 6
H1 = 16
G1 = NCH * H1  # 96
H2 = 128
T1_FULL = 1024
T2_FULL = 96
CH = 32  # stage-1 steps per staged x chunk
NST = G1 + NCH + 1  # 103 state rows: h | x | ones

# on-chip gate bank order: i, f, o, g ; torch rows are i, f, g, o
GATE_BASES_1 = (0, H1, 3 * H1, 2 * H1)
GATE_BASES_2 = (0, H2, 3 * H2, 2 * H2)

# packed weight column map
_off = 0


def _take(n):
    global _off
    o = _off
    _off += n
    return o


W1S_O = _take(4 * G1)  # stacked stage-1 weights, rows 0:103
W2X_O = _take(4 * H2)  # row 0 (x weights)
B2_O = _take(4 * H2)  # row 0 (stage-2 bias)
W2H_O = _take(4 * H2)
W3X_O = _take(4 * H2)
W3H_O = _take(4 * H2)
B3_O = _take(4 * H2)
WF1_O = _take(H2)
WF3_O = _take(H2)
WF2_O = _take(2)
BF1_O = _take(1)
BF3_O = _take(1)
BF2_O = _take(1)
ONES_O = _take(BC)
WCOLS = _off


def _r(ap, pattern, **kw):
    return ap.rearrange(pattern, **kw)


def build_program(T1=T1_FULL, T2=T2_FULL, ch=CH, staggered=False):
    assert T1 % (2 * ch) == 0
    nc = bacc.Bacc("TRN2", target_bir_lowering=False)

    tpad = 2 * ch
    x_t = nc.declare_dram_parameter("xT", [7, T1 + tpad, BC], F32R, isOutput=False)
    wp_d = nc.declare_dram_parameter("wpack", [128, WCOLS], F32R, isOutput=False)
    y_t = nc.declare_dram_parameter("yT", [2, BC], F32, isOutput=True)

    feat_d = nc.dram_tensor("featstage", [G1 + 32, BC], F32R)

    def mm(out, lhsT, rhs, start, stop):
        nc.tensor.matmul(out, lhsT, rhs, start=start, stop=stop)

    with tile.TileContext(nc) as tc:
        with (
            tc.tile_pool(name="wpool", bufs=1) as wpool,
            tc.tile_pool(name="state", bufs=1) as state,
            tc.tile_pool(name="work", bufs=3) as work,
        ):
            wp = wpool.tile([128, WCOLS], F32R)
            nc.sync.dma_start(wp[:], wp_d[:])

            w1s = [wp[0:NST, W1S_O + G1 * t : W1S_O + G1 * (t + 1)] for t in range(4)]
            w2x = [wp[0:1, W2X_O + H2 * t : W2X_O + H2 * (t + 1)] for t in range(4)]
            b2 = [wp[0:1, B2_O + H2 * t : B2_O + H2 * (t + 1)] for t in range(4)]
            w2h = [wp[0:H2, W2H_O + H2 * t : W2H_O + H2 * (t + 1)] for t in range(4)]
            w3x = [wp[0:H2, W3X_O + H2 * t : W3X_O + H2 * (t + 1)] for t in range(4)]
            w3h = [wp[0:H2, W3H_O + H2 * t : W3H_O + H2 * (t + 1)] for t in range(4)]
            b3 = [wp[0:1, B3_O + H2 * t : B3_O + H2 * (t + 1)] for t in range(4)]
            wf1 = wp[0:H2, WF1_O : WF1_O + H2]
            wf3 = wp[0:H2, WF3_O : WF3_O + H2]
            wf2 = wp[0:H2, WF2_O : WF2_O + 2]
            bf1 = wp[0:H2, BF1_O : BF1_O + 1].bitcast(F32)
            bf3 = wp[0:H2, BF3_O : BF3_O + 1].bitcast(F32)
            bf2 = wp[0:2, BF2_O : BF2_O + 1].bitcast(F32)
            ones1 = wp[0:1, ONES_O : ONES_O + BC]

            # ---- stage 1: two interleaved half-streams ----
            st = [
                [state.tile([NST, BH], F32R, name=f"st{s}_{p}") for p in range(2)]
                for s in range(2)
            ]
            c1 = [state.tile([G1, BH], F32, name=f"c1_{s}") for s in range(2)]
            for s in range(2):
                for p in range(2):
                    nc.vector.memset(st[s][p][:].bitcast(F32), 0.0)
                    # rows 96:103 -> 1.0 (aligned start); x copies later
                    # overwrite 96:102, leaving the ones row at 102
                    nc.vector.memset(st[s][p][G1:NST, :].bitcast(F32), 1.0)
                nc.vector.memset(c1[s][:], 0.0)

            with (
                tc.tile_pool(name="ps1pool", bufs=1, space="PSUM") as ps1pool,
                tc.tile_pool(name="xsb", bufs=1) as xsb,
            ):
                xstage = [
                    xsb.tile([NST, ch * BC], F32R, name=f"xst_{k}") for k in range(2)
                ]
                xv = _r(x_t[:], "c t b -> c (t b)")
                nc.sync.dma_start(
                    xstage[0][G1 : G1 + NCH, :], xv[0:NCH, 0 : ch * BC]
                )
                nc.sync.dma_start(
                    xstage[1][G1 : G1 + NCH, :], xv[0:NCH, ch * BC : 2 * ch * BC]
                )
                nchunks = T1 // ch

                def xview(k, jj, s):
                    return _r(xstage[k], "p (t b) -> p t b", b=BC)[
                        G1 : G1 + NCH, jj, s * BH : (s + 1) * BH
                    ]

                # prologue: x_0 into both ping-0 state tiles
                for s in range(2):
                    nc.gpsimd.tensor_copy(st[s][0][G1 : G1 + NCH, :], xview(0, 0, s))

                def s1_mm(s, p):
                    # per-gate psum tiles so each activation waits only on
                    # its own matmul (tile-granularity deps)
                    psg = ps1pool.tile([G1, 512], F32, name=f"psg{s}", tag=f"psg{s}")
                    psif = ps1pool.tile(
                        [G1, 1024], F32, name=f"psif{s}", tag=f"psif{s}"
                    )
                    pso = ps1pool.tile([G1, 512], F32, name=f"pso{s}", tag=f"pso{s}")
                    rhs = st[s][p][:]
                    mm(psg[:, 0:BH], w1s[3], rhs, True, True)
                    mm(psif[:, 0:BH], w1s[0], rhs, True, True)
                    mm(psif[:, 512 : 512 + BH], w1s[1], rhs, True, True)
                    mm(pso[:, 0:BH], w1s[2], rhs, True, True)
                    return psg, psif, pso

                def s1_cell(s, p, ps3, knext, jjnext):
                    psg, psif, pso = ps3
                    g_ = work.tile([G1, BH], F32, name=f"gt{s}", tag=f"gt{s}")
                    nc.scalar.activation(g_[:], psg[:, 0:BH], AF.Tanh)
                    sif_ = work.tile([G1, 2 * BH], F32, name=f"sif{s}", tag=f"sif{s}")
                    nc.scalar.activation(
                        _r(sif_, "p (t b) -> p t b", b=BH),
                        _r(psif, "p (t b) -> p t b", b=512)[:, 0:2, 0:BH],
                        AF.Sigmoid,
                    )
                    so_ = work.tile([G1, BH], F32, name=f"so{s}", tag=f"so{s}")
                    nc.scalar.activation(so_[:], pso[:, 0:BH], AF.Sigmoid)
                    u_ = work.tile([G1, BH], F32, name=f"u{s}", tag=f"u{s}")
                    nc.vector.tensor_mul(u_[:], sif_[:, 0:BH], g_[:])
                    tm_ = work.tile([G1, BH], F32, name=f"tm{s}", tag=f"tm{s}")
                    nc.gpsimd.tensor_mul(tm_[:], sif_[:, BH:], c1[s][:])
                    nc.vector.tensor_add(c1[s][:], u_[:], tm_[:])
                    th_ = work.tile([G1, BH], F32, name=f"th{s}", tag=f"th{s}")
                    nc.scalar.activation(th_[:], c1[s][:], AF.Tanh)
                    nc.vector.tensor_mul(st[s][1 - p][0:G1, :], so_[:], th_[:])
                    nc.gpsimd.tensor_copy(
                        st[s][1 - p][G1 : G1 + NCH, :], xview(knext, jjnext, s)
                    )

                # software-pipelined: stream B runs half a step behind A, so
                # B's matmuls overlap A's cell math and vice versa. The A-side
                # pipeline is primed/drained inside each body so no tile flows
                # across the hardware-loop boundary.
                def chunk_pair(ivc):
                    pendA = (0, s1_mm(0, 0))
                    for k in range(2):
                        for jj in range(ch):
                            if jj < ch - 1:
                                kn, jn = k, jj + 1
                            else:
                                kn, jn = 1 - k, 0
                            p = jj % 2
                            psB = s1_mm(1, p)
                            ap, aps = pendA
                            s1_cell(0, ap, aps, kn, jn)  # A cell, this step
                            if not (k == 1 and jj == ch - 1):
                                pendA = (1 - p, s1_mm(0, 1 - p))
                            s1_cell(1, p, psB, kn, jn)
                        pre = (ivc + (2 + k)) * (ch * BC)
                        nc.sync.dma_start(
                            xstage[k][G1 : G1 + NCH, :],
                            xv[0:NCH, bass.ds(pre, ch * BC)],
                        )

                chunk_pair(0)  # peeled: absorbs prologue DMA waits
                assert nchunks >= 4
                with tc.For_i(2, nchunks, 2, staggered_reset=staggered) as ivc:
                    chunk_pair(ivc)

                # final h -> feat  (last write was into ping T1%2)
                pf = T1 % 2
                nc.sync.dma_start(feat_d[0:G1, 0:BH], st[0][pf][0:G1, :])
                nc.sync.dma_start(feat_d[0:G1, BH:BC], st[1][pf][0:G1, :])

            # ---- stages 2 & 3, wavefronted ----
            with tc.tile_pool(name="psum", bufs=2, space="PSUM") as psum_pool:
                h2s = [state.tile([H2, BC], F32R, name=f"h2s_{p}") for p in range(2)]
                c2 = state.tile([H2, BC], F32)
                h3s = [state.tile([H2, BC], F32R, name=f"h3s_{p}") for p in range(2)]
                c3 = state.tile([H2, BC], F32)
                for t_ in h2s + h3s:
                    nc.vector.memset(t_[:].bitcast(F32), 0.0)
                nc.vector.memset(c2[:], 0.0)
                nc.vector.memset(c3[:], 0.0)
                x2blk = [
                    state.tile([1, 8 * BC], F32R, name=f"x2blk_{p}") for p in range(2)
                ]

                def cell23(ps, c, h_out, pfx):
                    gt = work.tile([H2, BC], F32, name=f"gt{pfx}", tag="gt23")
                    nc.scalar.activation(gt[:], ps[:, 3 * 512 : 3 * 512 + BC], AF.Tanh)
                    s_all = work.tile([H2, 3 * BC], F32, name=f"s{pfx}", tag="s23")
                    nc.scalar.activation(
                        _r(s_all, "p (t b) -> p t b", b=BC),
                        _r(ps, "p (t b) -> p t b", b=512)[:, 0:3, 0:BC],
                        AF.Sigmoid,
                    )
                    u = work.tile([H2, BC], F32, name=f"u{pfx}", tag="u23")
                    nc.vector.tensor_mul(u[:], s_all[:, 0:BC], gt[:])
                    tm = work.tile([H2, BC], F32, name=f"tm{pfx}", tag="tm23")
                    nc.gpsimd.tensor_mul(tm[:], s_all[:, BC : 2 * BC], c[:])
                    nc.vector.tensor_add(c[:], u[:], tm[:])
                    th = work.tile([H2, BC], F32, name=f"th{pfx}", tag="th23")
                    nc.scalar.activation(th[:], c[:], AF.Tanh)
                    nc.vector.tensor_mul(h_out[:], s_all[:, 2 * BC :], th[:])

                def step2(j, xrow):
                    ps = psum_pool.tile([128, 2048], F32, name="ps2", tag="ps")
                    for t in range(4):
                        mm(ps[:, t * 512 : t * 512 + BC], b2[t], ones1, True, False)
                    for t in range(4):
                        mm(ps[:, t * 512 : t * 512 + BC], w2x[t], xrow, False, False)
                    for t in (3, 0, 1, 2):
                        mm(
                            ps[:, t * 512 : t * 512 + BC],
                            w2h[t],
                            h2s[j % 2],
                            False,
                            True,
                        )
                    cell23(ps, c2, h2s[(j + 1) % 2], "2")

                def step3(j):
                    ps = psum_pool.tile([128, 2048], F32, name="ps3", tag="ps")
                    for t in range(4):
                        mm(ps[:, t * 512 : t * 512 + BC], b3[t], ones1, True, False)
                    for t in range(4):
                        mm(
                            ps[:, t * 512 : t * 512 + BC],
                            w3x[t],
                            h2s[(j + 1) % 2],
                            False,
                            False,
                        )
                    for t in (3, 0, 1, 2):
                        mm(ps[:, t * 512 : t * 512 + BC], w3h[t], h3s[j % 2], False, True)
                    cell23(ps, c3, h3s[(j + 1) % 2], "3")

                W2 = 16
                nc.sync.dma_start(x2blk[0][:], _r(feat_d[0:8, :], "r b -> (r b)"))
                nc.sync.dma_start(x2blk[1][:], _r(feat_d[8:16, :], "r b -> (r b)"))

                def wave_block(ivw):
                    for half in range(2):
                        for jj in range(8):
                            j = 8 * half + jj
                            step2(j, x2blk[half][0:1, jj * BC : (jj + 1) * BC])
                            step3(j)
                        off = ivw + 16 + 8 * half
                        nc.sync.dma_start(
                            x2blk[half][:],
                            _r(feat_d[bass.ds(off, 8), :], "r b -> (r b)"),
                        )

                wave_block(0)  # peeled
                with tc.For_i(W2, T2, W2, staggered_reset=staggered) as ivw:
                    wave_block(ivw)

                # ---- FC head ----
                h3f = h3s[T2 % 2]
                psf = psum_pool.tile([128, 2048], F32, name="psf", tag="ps")
                mm(psf[:, 0:BC], wf1, h3f, True, True)
                a1 = work.tile([H2, BC], F32R)
                nc.vector.tensor_scalar(
                    a1[:], psf[:, 0:BC], bf1, 0.0, op0=ALU.add, op1=ALU.max
                )
                mm(psf[:, 512 : 512 + BC], wf3, a1, True, True)
                a3 = work.tile([H2, BC], F32R)
                nc.vector.tensor_scalar(
                    a3[:], psf[:, 512 : 512 + BC], bf3, 0.0, op0=ALU.add, op1=ALU.max
                )
                mm(psf[0:2, 1024 : 1024 + BC], wf2, a3, True, True)
                yt = work.tile([2, BC], F32)
                nc.vector.tensor_scalar_add(yt[:], psf[0:2, 1024 : 1024 + BC], bf2)
                nc.sync.dma_start(y_t[:], yt[:])

    nc.compile()
    return nc


def tf32_round(a):
    u = np.ascontiguousarray(a, np.float32).view(np.uint32)
    u = (u + np.uint32(0x1000)) & np.uint32(0xFFFFE000)
    return u.view(np.float32)


def pack_weights(i):
    f32 = np.float32
    wp = np.zeros((128, WCOLS), f32)
    Wih, Whh = np.asarray(i["rnn_Wih"], f32), np.asarray(i["rnn_Whh"], f32)
    bb1 = np.asarray(i["rnn_bih"], f32) + np.asarray(i["rnn_bhh"], f32)
    for t, base in enumerate(GATE_BASES_1):
        o = W1S_O + G1 * t
        for c in range(NCH):
            # h-part rows 0:96 (block-diagonal per channel)
            wp[16 * c : 16 * c + 16, o + 16 * c : o + 16 * c + 16] = (
                Whh[base : base + H1, :].T
            )
            # x-part row 96+c
            wp[G1 + c, o + 16 * c : o + 16 * c + 16] = Wih[base : base + H1, 0]
        # bias row 102
        wp[G1 + NCH, o : o + G1] = np.tile(bb1[base : base + H1], NCH)
    bb2 = np.asarray(i["rnn2_bih0"], f32) + np.asarray(i["rnn2_bhh0"], f32)
    for t, base in enumerate(GATE_BASES_2):
        wp[0, W2X_O + H2 * t : W2X_O + H2 * (t + 1)] = np.asarray(
            i["rnn2_Wih0"], f32
        )[base : base + H2, 0]
        wp[0, B2_O + H2 * t : B2_O + H2 * (t + 1)] = bb2[base : base + H2]
        wp[0:H2, W2H_O + H2 * t : W2H_O + H2 * (t + 1)] = np.asarray(
            i["rnn2_Whh0"], f32
        )[base : base + H2, :].T
    bb3 = np.asarray(i["rnn2_bih1"], f32) + np.asarray(i["rnn2_bhh1"], f32)
    for t, base in enumerate(GATE_BASES_2):
        wp[0:H2, W3X_O + H2 * t : W3X_O + H2 * (t + 1)] = np.asarray(
            i["rnn2_Wih1"], f32
        )[base : base + H2, :].T
        wp[0:H2, W3H_O + H2 * t : W3H_O + H2 * (t + 1)] = np.asarray(
            i["rnn2_Whh1"], f32
        )[base : base + H2, :].T
        wp[0, B3_O + H2 * t : B3_O + H2 * (t + 1)] = bb3[base : base + H2]
    wp[0:H2, WF1_O : WF1_O + H2] = np.asarray(i["fc1_W"], f32).T
    wp[0:H2, WF3_O : WF3_O + H2] = np.asarray(i["fc3_W"], f32).T
    wp[0:H2, WF2_O : WF2_O + 2] = np.asarray(i["fc2_W"], f32).T
    # matmul-consumed columns get TF32 rounding
    wp = tf32_round(wp)
    # biases used as fp32 DVE scalars keep full precision
    wp[0:H2, BF1_O] = np.asarray(i["fc1_b"], f32)
    wp[0:H2, BF3_O] = np.asarray(i["fc3_b"], f32)
    wp[0:2, BF2_O] = np.asarray(i["fc2_b"], f32)
    wp[0, ONES_O : ONES_O + BC] = 1.0
    return wp


def make_in_maps(inputs, T1=T1_FULL, ch=CH):
    wp = pack_weights(inputs)
    x = np.asarray(inputs["x"], np.float32)
    tpad = 2 * ch
    maps = []
    for k in range(NCORES):
        xk = np.zeros((7, T1 + tpad, BC), np.float32)
        xk[0:6, :T1, :] = tf32_round(
            np.ascontiguousarray(x[k * BC : (k + 1) * BC, :T1, :].transpose(2, 1, 0))
        )
        xk[6, :, :] = 1.0
        maps.append({"xT": xk, "wpack": wp})
    return maps


def kernel(**inputs):
    from concourse.bass_utils import run_bass_kernel_spmd

    nc = build_program()
    in_maps = make_in_maps(inputs)
    res = run_bass_kernel_spmd(nc, in_maps, list(range(NCORES)))
    outs = [np.asarray(res.results[k]["yT"]) for k in range(NCORES)]
    return np.concatenate([o.T for o in outs], axis=0).astype(np.float32)
